# revision 32
# baseline (speedup 1.0000x reference)
"""Trainium2 Bass kernel for nn_Encoder_79585743995180 (sparse_attention).

Self-contained: hardcodes shapes/sharding. Strategy (validated in numpy):
  - 8 cores, head-parallel: core c owns heads {2c, 2c+1} (128 of 1024 dims).
  - Per core: q/k/v projections for its 128 dims (reads full activations,
    sliced weights), rope (de-interleaved even/odd permutation so the
    rotation partner sits at partition offset +32 within each 64-dim head
    block), main attention with column-softmax folded into a 1/colsum
    prescale of the AV stationary operand, memory attention with mask+gate
    folded into the host-prepped vmaug tensor, out_proj partial product.
  - Host sums the 8 partial outputs (contraction-sharded out_proj).
  - Matmul operands in fp16; accumulation fp32 in PSUM; softmax
    renormalization path fp32.

Hardware findings baked in (micro-benchmarked):
  - engine ops may shift the OUTPUT partition offset but all source APs
    must sit on the same partitions, and outputs must start on 32-aligned
    partitions; reciprocal_approx_fast only works at partition 0; gpsimd
    cannot touch PSUM and has ~1-2us fixed cost per op (DMA-issue only).
  - scalar ACTIVATE costs ~(cols+352)/1.2 ns, so score tiles are
    [128,1024] (both L-halves of one s-chunk) and softmax uses ONE exp +
    one accumulator read per s-chunk: colsum lands as a [128,1] side
    effect, no separate reductions anywhere.
  - main attention drains PSUM to SBUF so the memory attention reuses the
    same accumulator banks; this frees PSUM for the wide score tiles.
  - renorm divide: denominator rows (row 64 of the accumulators) are
    copied to partition 0, inverted with reciprocal_approx_fast, bounced
    through DRAM (SBUF stride-0 source DMAs are illegal) and broadcast,
    then two muls + add. No 6.5us single-partition reciprocal.
"""

import ml_dtypes
import numpy as np

import concourse.bacc as bacc
import concourse.mybir as mybir
import concourse.tile as tile
from concourse import bass_utils

F32 = mybir.dt.float32
BF16 = mybir.dt.float16
NPBF = np.float16
AF = mybir.ActivationFunctionType

L = 1024
S = 1024
N = 4
E = 1024
H = 16
D = 64
M = 512
NC = 8
HPC = H // NC          # 2 heads per core
DC = HPC * D           # 128 dims per core
R = L * N              # 4096 rows, r = n*L + l

_COMPILED = {}


def _build(dbg=False):
    nc = bacc.Bacc("TRN2", target_bir_lowering=False, debug=False)

    # ---- DRAM I/O ----
    xqT = nc.dram_tensor("xqT", [E, R], BF16, kind="ExternalInput").ap()
    xkT = nc.dram_tensor("xkT", [E, R], BF16, kind="ExternalInput").ap()
    xvT = nc.dram_tensor("xvT", [E, R], BF16, kind="ExternalInput").ap()
    wqT = nc.dram_tensor("wqT", [E, DC], BF16, kind="ExternalInput").ap()
    wkT = nc.dram_tensor("wkT", [E, DC], BF16, kind="ExternalInput").ap()
    wvT = nc.dram_tensor("wvT", [E, DC], BF16, kind="ExternalInput").ap()
    woT = nc.dram_tensor("woT", [DC, E], BF16, kind="ExternalInput").ap()
    cosq = nc.dram_tensor("cosq", [DC, R], BF16, kind="ExternalInput").ap()
    sinq = nc.dram_tensor("sinq", [DC, R], BF16, kind="ExternalInput").ap()
    cosk = nc.dram_tensor("cosk", [DC, R], BF16, kind="ExternalInput").ap()
    sink = nc.dram_tensor("sink", [DC, R], BF16, kind="ExternalInput").ap()
    kmem = nc.dram_tensor("kmem", [DC, N, M], BF16, kind="ExternalInput").ap()
    vmaug = nc.dram_tensor("vmaug", [128, N, HPC, 4, 65], BF16,
                           kind="ExternalInput").ap()
    outT = nc.dram_tensor("outT", [E, R], BF16, kind="ExternalOutput").ap()
    dbg_t = {}
    if dbg:
        for nm, shp in (("dbg_q", [DC, R]), ("dbg_k", [DC, R]),
                        ("dbg_attn", [DC, R])):
            dbg_t[nm] = nc.dram_tensor(nm, shp, F32, kind="ExternalOutput").ap()

    with tile.TileContext(nc) as tc:
        with (
            tc.tile_pool(name="const", bufs=1) as const,
            tc.tile_pool(name="persist", bufs=1) as persist,
            tc.tile_pool(name="xstream", bufs=3) as xstream,
            tc.tile_pool(name="cs", bufs=2) as cs,
            tc.tile_pool(name="ropescr", bufs=3) as ropescr,
            tc.tile_pool(name="wexp", bufs=14) as wexpp,
            tc.tile_pool(name="small", bufs=3) as small,
            tc.tile_pool(name="renorm", bufs=2) as renorm,
            tc.tile_pool(name="renorm1", bufs=1) as renorm1,
            tc.tile_pool(name="drows", bufs=3, space="DRAM") as drows,
            tc.tile_pool(name="ostage", bufs=4) as ostage,
            tc.tile_pool(name="pw", bufs=2, space="PSUM") as pw,
            tc.tile_pool(name="pproj", bufs=2, space="PSUM") as pproj,
            tc.tile_pool(name="pacc", bufs=1, space="PSUM") as pacc,
        ):
            # ---- weights into SBUF (k first: k-proj leads; per-kc
            # chunks so the first matmul only waits for 32KB) ----
            w_sb = {}
            for name, src in (("k", wkT), ("q", wqT), ("v", wvT)):
                t = const.tile([128, 8, DC], BF16, tag=f"w_{name}",
                               name=f"w_{name}")
                for kc in range(8):
                    nc.sync.dma_start(
                        out=t[:, kc, :],
                        in_=src[kc * 128:(kc + 1) * 128, :])
                w_sb[name] = t

            qT_n = [persist.tile([DC, L], BF16, tag=f"qT{n}", name=f"qT{n}")
                    for n in range(N)]
            kT_n = [persist.tile([DC, L], BF16, tag=f"kT{n}", name=f"kT{n}")
                    for n in range(N)]
            v_n = [persist.tile([128, 8, HPC, 65], BF16, tag=f"v{n}",
                                name=f"v{n}") for n in range(N)]
            attn_n = [persist.tile([DC, L], BF16, tag=f"at{n}",
                                   name=f"at{n}") for n in range(N)]
            for n in range(N):
                nc.vector.memset(v_n[n][:, :, :, 64:65], 1.0)

            def emit_proj(n):
                # ---- projections for batch n (rows n*L .. n*L+L) ----
                nrows = slice(n * L, (n + 1) * L)
                for name, xT, cosT, sinT in (
                    ("k", xkT, cosk, sink),
                    ("q", xqT, cosq, sinq),
                ):
                    dest = qT_n[n] if name == "q" else kT_n[n]
                    xs = xstream.tile([128, 8, 1024], BF16, tag="xs")
                    for kc in range(8):
                        dq = (nc.scalar, nc.gpsimd, nc.sync)[
                            (kc + (0 if name == "k" else 1)) % 3]
                        dq.dma_start(
                            out=xs[:, kc, :],
                            in_=xT[kc * 128:(kc + 1) * 128, nrows])
                    ctw = cs.tile([128, 1024], BF16, tag="ct")
                    stw = cs.tile([128, 1024], BF16, tag="st")
                    nc.sync.dma_start(out=ctw, in_=cosT[:, nrows])
                    nc.sync.dma_start(out=stw, in_=sinT[:, nrows])
                    for rt2 in range(2):
                        ls = slice(rt2 * 512, (rt2 + 1) * 512)
                        ps = pproj.tile([128, 512], F32, tag="pp")
                        for kc in range(8):
                            nc.tensor.matmul(
                                ps, w_sb[name][:, kc, :], xs[:, kc, ls],
                                start=(kc == 0), stop=(kc == 7))
                        # stage once on scalar (frees the PSUM slot via two
                        # parallel engines), then the sin product lands in
                        # the partner rows via output-shifted quarter muls
                        psb = ropescr.tile([128, 512], BF16, tag="psb")
                        nc.scalar.activation(psb, ps, AF.Copy)
                        t1 = ropescr.tile([128, 512], BF16, tag="t1")
                        nc.vector.tensor_mul(t1, ps, ctw[:, ls])
                        zsw = ropescr.tile([128, 512], BF16, tag="zsw")
                        for qb in range(4):
                            b = qb * 32
                            pb = (qb ^ 1) * 32   # partner block
                            nc.vector.tensor_mul(
                                zsw[pb:pb + 32, :],
                                psb[b:b + 32, :], stw[b:b + 32, ls])
                        nc.vector.tensor_add(dest[:, ls], t1, zsw)
                # v projection for batch n (row-major: x rows as stationary)
                xs = xstream.tile([128, 8, 1024], BF16, tag="xs")
                for kc in range(8):
                    nc.sync.dma_start(
                        out=xs[:, kc, :],
                        in_=xvT[kc * 128:(kc + 1) * 128, nrows])
                for st_i in range(8):
                    ps = pproj.tile([128, 512], F32, tag="pp")
                    for kc in range(8):
                        nc.tensor.matmul(
                            ps[:, 0:128],
                            xs[:, kc, st_i * 128:(st_i + 1) * 128],
                            w_sb["v"][:, kc, :],
                            start=(kc == 0), stop=(kc == 7))
                    for h in range(HPC):
                        nc.vector.tensor_scalar_add(
                            v_n[n][:, st_i, h, 0:64],
                            ps[:, h * 64:(h + 1) * 64], 0.0)

            def emit_attn_out(n):
                # ---- attention for batch n, both heads ----
                for h in range(HPC):
                    ho = h * 64
                    colsum = small.tile([128, 8], F32, tag="colsum")
                    rcall = small.tile([128, 8], F32, tag="rcall")
                    pmain = pacc.tile([65, 1024], F32, tag="pacc")
                    wxs = []
                    vss = []

                    def emit_qk(sc):
                        pwt = pw.tile([128, 1024], F32, tag="pw")
                        for lc in range(2):
                            nc.tensor.matmul(
                                pwt[:, lc * 512:(lc + 1) * 512],
                                kT_n[n][ho:ho + 64,
                                        sc * 128:(sc + 1) * 128],
                                qT_n[n][ho:ho + 64,
                                        lc * 512:(lc + 1) * 512],
                                start=True, stop=True)
                        wx = wexpp.tile([128, 1024], BF16, tag="wx")
                        nc.scalar.activation(
                            wx, pwt, AF.Exp, accum_out=colsum[:, sc:sc + 1])
                        wxs.append(wx)
                        # per-sc renormalizer: AV(sc) only needs this chunk's
                        # column sum, so don't barrier on all 8 exps
                        nc.vector.reciprocal(
                            rcall[:, sc:sc + 1], colsum[:, sc:sc + 1])
                        vs = small.tile([128, 65], BF16, tag="vs")
                        nc.vector.tensor_scalar_mul(
                            vs, v_n[n][:, sc, h, :], rcall[:, sc:sc + 1])
                        vss.append(vs)

                    def emit_av(sc):
                        for lc in range(2):
                            nc.tensor.matmul(
                                pmain[:, lc * 512:(lc + 1) * 512],
                                vss[sc], wxs[sc][:, lc * 512:(lc + 1) * 512],
                                start=(sc == 0), stop=(sc == 7))

                    # interleave: AV(sc-1) fills the PE while exp(sc) drains
                    for sc in range(8):
                        emit_qk(sc)
                        if sc >= 1:
                            emit_av(sc - 1)
                    emit_av(7)
                    # drain main-attention accumulator to SBUF so the same
                    # PSUM banks take the memory attention (frees PSUM for
                    # the [128,1024] score tiles)
                    smain = renorm.tile([65, 1024], F32, tag="smain")
                    nc.vector.tensor_scalar_add(smain, pmain, 0.0)
                    pmem = pacc.tile([65, 1024], F32, tag="pacc")
                    for mc in range(4):
                        pwt = pw.tile([128, 1024], F32, tag="pw")
                        for lc in range(2):
                            nc.tensor.matmul(
                                pwt[:, lc * 512:(lc + 1) * 512],
                                kmem_sb[ho:ho + 64, n,
                                        mc * 128:(mc + 1) * 128],
                                qT_n[n][ho:ho + 64,
                                        lc * 512:(lc + 1) * 512],
                                start=True, stop=True)
                        wxm = wexpp.tile([128, 1024], BF16, tag="wx")
                        nc.scalar.activation(wxm, pwt, AF.Exp)
                        for lc in range(2):
                            nc.tensor.matmul(
                                pmem[:, lc * 512:(lc + 1) * 512],
                                vmaug_sb[:, n, h, mc, :],
                                wxm[:, lc * 512:(lc + 1) * 512],
                                start=(mc == 0), stop=(mc == 3))
                    # renorm: out = smain[0:64]/D1 + pmem[0:64]/D2 where
                    # D1 = smain row 64, D2 = pmem row 64. Both rows go to
                    # partition 0 (shifted OUTPUT is legal), one reciprocal,
                    # one DRAM bounce for the partition broadcast.
                    dd = renorm1.tile([1, 2048], F32, tag="dd")
                    nc.vector.tensor_scalar_add(
                        dd[:, 0:1024], smain[64:65, :], 0.0)
                    nc.scalar.activation(
                        dd[:, 1024:2048], pmem[64:65, :], AF.Copy)
                    rrf = renorm1.tile([1, 2048], F32, tag="rrf")
                    nc.vector.reciprocal_approx_fast(rrf, dd)
                    rrh = renorm1.tile([1, 2048], BF16, tag="rrh")
                    nc.vector.tensor_scalar_add(rrh, rrf, 0.0)
                    drr = drows.tile([1, 2048], BF16, tag="drr")
                    nc.gpsimd.dma_start(out=drr, in_=rrh)
                    bc12 = renorm.tile([64, 2048], BF16, tag="bc12")
                    nc.sync.dma_start(
                        out=bc12, in_=drr.to_broadcast((64, 2048)))
                    u1 = renorm.tile([64, 1024], BF16, tag="u1")
                    nc.vector.tensor_mul(u1, smain[0:64, :], bc12[:, 0:1024])
                    u2 = renorm.tile([64, 1024], BF16, tag="u2")
                    nc.vector.tensor_mul(u2, pmem[0:64, :], bc12[:, 1024:2048])
                    nc.vector.tensor_add(attn_n[n][ho:ho + 64, :], u1, u2)

                if dbg:
                    nc.sync.dma_start(
                        out=dbg_t["dbg_q"][:, n * L:(n + 1) * L], in_=qT_n[n])
                    nc.sync.dma_start(
                        out=dbg_t["dbg_k"][:, n * L:(n + 1) * L], in_=kT_n[n])
                    nc.sync.dma_start(
                        out=dbg_t["dbg_attn"][:, n * L:(n + 1) * L],
                        in_=attn_n[n])

                # ---- out_proj partial for batch n ----
                for oc in range(8):
                    for rt2 in range(2):
                        po = pproj.tile([128, 512], F32, tag="pp")
                        nc.tensor.matmul(
                            po, wo_sb[:, oc * 128:(oc + 1) * 128],
                            attn_n[n][:, rt2 * 512:(rt2 + 1) * 512],
                            start=True, stop=True)
                        so = ostage.tile([128, 512], BF16, tag="so")
                        nc.vector.tensor_scalar_add(so, po, 0.0)
                        dst = outT[oc * 128:(oc + 1) * 128,
                                   n * L + rt2 * 512:n * L + (rt2 + 1) * 512]
                        if (oc * 2 + rt2) % 2 == 0:
                            nc.sync.dma_start(out=dst, in_=so)
                        else:
                            nc.gpsimd.dma_start(out=dst, in_=so)

            emit_proj(0)
            wo_sb = const.tile([DC, E], BF16)
            nc.sync.dma_start(out=wo_sb, in_=woT)
            kmem_sb = const.tile([DC, N, M], BF16)
            nc.gpsimd.dma_start(out=kmem_sb, in_=kmem)
            vmaug_sb = const.tile([128, N, HPC, 4, 65], BF16)
            nc.gpsimd.dma_start(out=vmaug_sb, in_=vmaug)
            for n in range(N):
                if n + 1 < N:
                    emit_proj(n + 1)
                emit_attn_out(n)
    nc.compile()
    return nc


def _perm64():
    p = np.empty(64, np.int64)
    p[:32] = np.arange(0, 64, 2)
    p[32:] = np.arange(1, 64, 2)
    return p


def _prep_inputs(inputs):
    """Host-side shard prep. Returns list of per-core input dicts."""
    f = np.float32
    query = np.asarray(inputs["query"], f)
    key = np.asarray(inputs["key"], f)
    value = np.asarray(inputs["value"], f)
    W = np.asarray(inputs["in_proj_weight"], f)
    wo = np.asarray(inputs["out_proj_weight"], f)
    qp = np.asarray(inputs["qp"], f)
    kvp = np.asarray(inputs["kvp"], f)
    k_mem = np.asarray(inputs["k_mem"], f)
    v_mem = np.asarray(inputs["v_mem"], f)
    gate = np.asarray(inputs["gate_attn"], f)
    mask = np.asarray(inputs["mem_mask"]).astype(f)

    g = 1.0 / (1.0 + np.exp(-gate))
    perm64 = _perm64()
    sgn = np.concatenate([np.full(32, -1.0, f), np.full(32, 1.0, f)] * HPC)

    xqT = np.ascontiguousarray(
        query.transpose(2, 1, 0).reshape(E, R)).astype(NPBF)
    xkT = np.ascontiguousarray(
        key.transpose(2, 1, 0).reshape(E, R)).astype(NPBF)
    xvT = np.ascontiguousarray(
        value.transpose(2, 1, 0).reshape(E, R)).astype(NPBF)

    in_maps = []
    for c in range(NC):
        dims = np.arange(c * DC, (c + 1) * DC)
        dims_perm = np.concatenate([dims[h * 64 + perm64] for h in range(HPC)])
        gv = np.concatenate(
            [np.full(64, 1.0 - g[2 * c + h], f) for h in range(HPC)])

        wq = W[:E][dims_perm] * np.float32(D ** -0.5)
        wk = W[E:2 * E][dims_perm]
        wv = W[2 * E:][dims] * gv[:, None]

        def rope(pe):
            cosT = np.ascontiguousarray(
                pe[:, :, dims_perm, 0].transpose(2, 0, 1).reshape(DC, R))
            sinT = (pe[:, :, dims_perm, 1].transpose(2, 0, 1).reshape(DC, R)
                    * sgn[:, None])
            # device writes z into the partner rows, so the sin tensor must
            # be pre-swapped: st[p] = sin_signed[partner(p)]
            sw = np.empty_like(sinT)
            for hb in range(HPC):
                b = hb * 64
                sw[b:b + 32] = sinT[b + 32:b + 64]
                sw[b + 32:b + 64] = sinT[b:b + 32]
            return cosT.astype(NPBF), np.ascontiguousarray(sw).astype(NPBF)

        cq, sq = rope(qp)
        ck, sk = rope(kvp)

        kmemT = np.ascontiguousarray(
            k_mem[:, dims_perm, :].transpose(1, 0, 2)).astype(NPBF)

        vma = np.zeros((N, HPC, M, 65), f)
        for n in range(N):
            for h in range(HPC):
                gh = g[2 * c + h]
                vm = v_mem[n, dims[h * 64:(h + 1) * 64], :].T  # (M, 64)
                vma[n, h, :, :64] = vm * gh * mask[n][:, None]
                vma[n, h, :, 64] = mask[n]
        vma_dev = np.ascontiguousarray(
            vma.reshape(N, HPC, 4, 128, 65).transpose(3, 0, 1, 2, 4)).astype(NPBF)

        in_maps.append({
            "xqT": xqT, "xkT": xkT, "xvT": xvT,
            "wqT": np.ascontiguousarray(wq.T).astype(NPBF),
            "wkT": np.ascontiguousarray(wk.T).astype(NPBF),
            "wvT": np.ascontiguousarray(wv.T).astype(NPBF),
            "woT": np.ascontiguousarray(wo[:, dims].T).astype(NPBF),
            "cosq": cq, "sinq": sq, "cosk": ck, "sink": sk,
            "kmem": kmemT, "vmaug": vma_dev,
        })
    return in_maps


def kernel(**inputs):
    if "nc" not in _COMPILED:
        _COMPILED["nc"] = _build()
    nc = _COMPILED["nc"]
    in_maps = _prep_inputs(inputs)
    res = bass_utils.run_bass_kernel_spmd(nc, in_maps, core_ids=list(range(NC)))
    total = np.zeros((E, R), np.float64)
    for r in res.results:
        total += r["outT"].astype(np.float64)
    out = total.T.reshape(N, L, E).transpose(1, 0, 2).astype(np.float32)
    out = out + np.asarray(inputs["out_proj_bias"], np.float32)
    return out


# revision 33
# speedup vs baseline: 1.0344x; 1.0344x over previous
"""Trainium2 Bass kernel for nn_Encoder_79585743995180 (sparse_attention).

Self-contained: hardcodes shapes/sharding. Strategy (validated in numpy):
  - 8 cores, head-parallel: core c owns heads {2c, 2c+1} (128 of 1024 dims).
  - Per core: q/k/v projections for its 128 dims (reads full activations,
    sliced weights), rope (de-interleaved even/odd permutation so the
    rotation partner sits at partition offset +32 within each 64-dim head
    block), main attention with column-softmax folded into a 1/colsum
    prescale of the AV stationary operand, memory attention with mask+gate
    folded into the host-prepped vmaug tensor, out_proj partial product.
  - Host sums the 8 partial outputs (contraction-sharded out_proj).
  - Matmul operands in fp16; accumulation fp32 in PSUM; softmax
    renormalization path fp32.

Hardware findings baked in (micro-benchmarked):
  - engine ops may shift the OUTPUT partition offset but all source APs
    must sit on the same partitions, and outputs must start on 32-aligned
    partitions; reciprocal_approx_fast only works at partition 0; gpsimd
    cannot touch PSUM and has ~1-2us fixed cost per op (DMA-issue only).
  - scalar ACTIVATE costs ~(cols+352)/1.2 ns, so score tiles are
    [128,1024] (both L-halves of one s-chunk) and softmax uses ONE exp +
    one accumulator read per s-chunk: colsum lands as a [128,1] side
    effect, no separate reductions anywhere.
  - main attention drains PSUM to SBUF so the memory attention reuses the
    same accumulator banks; this frees PSUM for the wide score tiles.
  - renorm divide: denominator rows (row 64 of the accumulators) are
    copied to partition 0, inverted with reciprocal_approx_fast, bounced
    through DRAM (SBUF stride-0 source DMAs are illegal) and broadcast,
    then two muls + add. No 6.5us single-partition reciprocal.
"""

import ml_dtypes
import numpy as np

import concourse.bacc as bacc
import concourse.mybir as mybir
import concourse.tile as tile
from concourse import bass_utils

F32 = mybir.dt.float32
BF16 = mybir.dt.float16
NPBF = np.float16
AF = mybir.ActivationFunctionType

L = 1024
S = 1024
N = 4
E = 1024
H = 16
D = 64
M = 512
NC = 8
HPC = H // NC          # 2 heads per core
DC = HPC * D           # 128 dims per core
R = L * N              # 4096 rows, r = n*L + l

_COMPILED = {}


def _build(dbg=False):
    nc = bacc.Bacc("TRN2", target_bir_lowering=False, debug=False)

    # ---- DRAM I/O ----
    xqT = nc.dram_tensor("xqT", [E, R], BF16, kind="ExternalInput").ap()
    xkT = nc.dram_tensor("xkT", [E, R], BF16, kind="ExternalInput").ap()
    xvT = nc.dram_tensor("xvT", [E, R], BF16, kind="ExternalInput").ap()
    wqT = nc.dram_tensor("wqT", [E, DC], BF16, kind="ExternalInput").ap()
    wkT = nc.dram_tensor("wkT", [E, DC], BF16, kind="ExternalInput").ap()
    wvT = nc.dram_tensor("wvT", [E, DC], BF16, kind="ExternalInput").ap()
    woT = nc.dram_tensor("woT", [DC, E], BF16, kind="ExternalInput").ap()
    cosq = nc.dram_tensor("cosq", [DC, R], BF16, kind="ExternalInput").ap()
    sinq = nc.dram_tensor("sinq", [DC, R], BF16, kind="ExternalInput").ap()
    cosk = nc.dram_tensor("cosk", [DC, R], BF16, kind="ExternalInput").ap()
    sink = nc.dram_tensor("sink", [DC, R], BF16, kind="ExternalInput").ap()
    kmem = nc.dram_tensor("kmem", [DC, N, M], BF16, kind="ExternalInput").ap()
    vmaug = nc.dram_tensor("vmaug", [128, N, HPC, 4, 65], BF16,
                           kind="ExternalInput").ap()
    outT = nc.dram_tensor("outT", [E, R], BF16, kind="ExternalOutput").ap()
    dbg_t = {}
    if dbg:
        for nm, shp in (("dbg_q", [DC, R]), ("dbg_k", [DC, R]),
                        ("dbg_attn", [DC, R])):
            dbg_t[nm] = nc.dram_tensor(nm, shp, F32, kind="ExternalOutput").ap()

    with tile.TileContext(nc) as tc:
        with (
            tc.tile_pool(name="const", bufs=1) as const,
            tc.tile_pool(name="persist", bufs=1) as persist,
            tc.tile_pool(name="xstream", bufs=3) as xstream,
            tc.tile_pool(name="cs", bufs=2) as cs,
            tc.tile_pool(name="ropescr", bufs=3) as ropescr,
            tc.tile_pool(name="wexp", bufs=14) as wexpp,
            tc.tile_pool(name="small", bufs=3) as small,
            tc.tile_pool(name="renorm", bufs=2) as renorm,
            tc.tile_pool(name="renorm1", bufs=1) as renorm1,
            tc.tile_pool(name="drows", bufs=3, space="DRAM") as drows,
            tc.tile_pool(name="ostage", bufs=4) as ostage,
            tc.tile_pool(name="pw", bufs=2, space="PSUM") as pw,
            tc.tile_pool(name="pproj", bufs=2, space="PSUM") as pproj,
            tc.tile_pool(name="pacc", bufs=1, space="PSUM") as pacc,
        ):
            # ---- weights into SBUF (k first: k-proj leads) ----
            w_sb = {}
            for name, src in (("k", wkT), ("q", wqT), ("v", wvT)):
                t = const.tile([128, 8, DC], BF16, tag=f"w_{name}",
                               name=f"w_{name}")
                nc.sync.dma_start(
                    out=t, in_=src.rearrange("(kc p) d -> p kc d", p=128))
                w_sb[name] = t

            qT_n = [persist.tile([DC, L], BF16, tag=f"qT{n}", name=f"qT{n}")
                    for n in range(N)]
            kT_n = [persist.tile([DC, L], BF16, tag=f"kT{n}", name=f"kT{n}")
                    for n in range(N)]
            v_n = [persist.tile([128, 8, HPC, 65], BF16, tag=f"v{n}",
                                name=f"v{n}") for n in range(N)]
            attn_n = [persist.tile([DC, L], BF16, tag=f"at{n}",
                                   name=f"at{n}") for n in range(N)]
            for n in range(N):
                nc.vector.memset(v_n[n][:, :, :, 64:65], 1.0)

            def emit_proj(n):
                # ---- projections for batch n (rows n*L .. n*L+L) ----
                nrows = slice(n * L, (n + 1) * L)
                for name, xT, cosT, sinT in (
                    ("k", xkT, cosk, sink),
                    ("q", xqT, cosq, sinq),
                ):
                    dest = qT_n[n] if name == "q" else kT_n[n]
                    xs = xstream.tile([128, 8, 1024], BF16, tag="xs")
                    for kc in range(8):
                        dq = nc.scalar if (kc % 2 == 0) == (name == "k") \
                            else nc.gpsimd
                        dq.dma_start(
                            out=xs[:, kc, :],
                            in_=xT[kc * 128:(kc + 1) * 128, nrows])
                    ctw = cs.tile([128, 1024], BF16, tag="ct")
                    stw = cs.tile([128, 1024], BF16, tag="st")
                    nc.sync.dma_start(out=ctw, in_=cosT[:, nrows])
                    nc.sync.dma_start(out=stw, in_=sinT[:, nrows])
                    for rt2 in range(2):
                        ls = slice(rt2 * 512, (rt2 + 1) * 512)
                        ps = pproj.tile([128, 512], F32, tag="pp")
                        for kc in range(8):
                            nc.tensor.matmul(
                                ps, w_sb[name][:, kc, :], xs[:, kc, ls],
                                start=(kc == 0), stop=(kc == 7))
                        # stage once on scalar (frees the PSUM slot via two
                        # parallel engines), then the sin product lands in
                        # the partner rows via output-shifted quarter muls
                        psb = ropescr.tile([128, 512], BF16, tag="psb")
                        nc.scalar.activation(psb, ps, AF.Copy)
                        t1 = ropescr.tile([128, 512], BF16, tag="t1")
                        nc.vector.tensor_mul(t1, ps, ctw[:, ls])
                        zsw = ropescr.tile([128, 512], BF16, tag="zsw")
                        for qb in range(4):
                            b = qb * 32
                            pb = (qb ^ 1) * 32   # partner block
                            nc.vector.tensor_mul(
                                zsw[pb:pb + 32, :],
                                psb[b:b + 32, :], stw[b:b + 32, ls])
                        nc.vector.tensor_add(dest[:, ls], t1, zsw)
                # v projection for batch n (row-major: x rows as stationary)
                xs = xstream.tile([128, 8, 1024], BF16, tag="xs")
                for kc in range(8):
                    nc.sync.dma_start(
                        out=xs[:, kc, :],
                        in_=xvT[kc * 128:(kc + 1) * 128, nrows])
                for st_i in range(8):
                    ps = pproj.tile([128, 512], F32, tag="pp")
                    for kc in range(8):
                        nc.tensor.matmul(
                            ps[:, 0:128],
                            xs[:, kc, st_i * 128:(st_i + 1) * 128],
                            w_sb["v"][:, kc, :],
                            start=(kc == 0), stop=(kc == 7))
                    for h in range(HPC):
                        nc.vector.tensor_scalar_add(
                            v_n[n][:, st_i, h, 0:64],
                            ps[:, h * 64:(h + 1) * 64], 0.0)

            def emit_attn_out(n):
                # ---- attention for batch n, both heads ----
                for h in range(HPC):
                    ho = h * 64
                    colsum = small.tile([128, 8], F32, tag="colsum")
                    wxs = []
                    for sc in range(8):
                        pwt = pw.tile([128, 1024], F32, tag="pw")
                        for lc in range(2):
                            nc.tensor.matmul(
                                pwt[:, lc * 512:(lc + 1) * 512],
                                kT_n[n][ho:ho + 64,
                                        sc * 128:(sc + 1) * 128],
                                qT_n[n][ho:ho + 64,
                                        lc * 512:(lc + 1) * 512],
                                start=True, stop=True)
                        wx = wexpp.tile([128, 1024], BF16, tag="wx")
                        nc.scalar.activation(
                            wx, pwt, AF.Exp, accum_out=colsum[:, sc:sc + 1])
                        wxs.append(wx)
                    rcall = small.tile([128, 8], F32, tag="rcall")
                    nc.vector.reciprocal(rcall, colsum)
                    pmain = pacc.tile([65, 1024], F32, tag="pacc")
                    for sc in range(8):
                        vs = small.tile([128, 65], BF16, tag="vs")
                        nc.vector.tensor_scalar_mul(
                            vs, v_n[n][:, sc, h, :], rcall[:, sc:sc + 1])
                        for lc in range(2):
                            nc.tensor.matmul(
                                pmain[:, lc * 512:(lc + 1) * 512],
                                vs, wxs[sc][:, lc * 512:(lc + 1) * 512],
                                start=(sc == 0), stop=(sc == 7))
                    # drain main-attention accumulator to SBUF so the same
                    # PSUM banks take the memory attention (frees PSUM for
                    # the [128,1024] score tiles)
                    smain = renorm.tile([65, 1024], F32, tag="smain")
                    nc.vector.tensor_scalar_add(smain, pmain, 0.0)
                    pmem = pacc.tile([65, 1024], F32, tag="pacc")
                    for mc in range(4):
                        pwt = pw.tile([128, 1024], F32, tag="pw")
                        for lc in range(2):
                            nc.tensor.matmul(
                                pwt[:, lc * 512:(lc + 1) * 512],
                                kmem_sb[ho:ho + 64, n,
                                        mc * 128:(mc + 1) * 128],
                                qT_n[n][ho:ho + 64,
                                        lc * 512:(lc + 1) * 512],
                                start=True, stop=True)
                        wxm = wexpp.tile([128, 1024], BF16, tag="wx")
                        nc.scalar.activation(wxm, pwt, AF.Exp)
                        for lc in range(2):
                            nc.tensor.matmul(
                                pmem[:, lc * 512:(lc + 1) * 512],
                                vmaug_sb[:, n, h, mc, :],
                                wxm[:, lc * 512:(lc + 1) * 512],
                                start=(mc == 0), stop=(mc == 3))
                    # renorm: out = smain[0:64]/D1 + pmem[0:64]/D2 where
                    # D1 = smain row 64, D2 = pmem row 64. Both rows go to
                    # partition 0 (shifted OUTPUT is legal), one reciprocal,
                    # one DRAM bounce for the partition broadcast.
                    dd = renorm1.tile([1, 2048], F32, tag="dd")
                    nc.vector.tensor_scalar_add(
                        dd[:, 0:1024], smain[64:65, :], 0.0)
                    nc.scalar.activation(
                        dd[:, 1024:2048], pmem[64:65, :], AF.Copy)
                    rrf = renorm1.tile([1, 2048], F32, tag="rrf")
                    nc.vector.reciprocal_approx_fast(rrf, dd)
                    rrh = renorm1.tile([1, 2048], BF16, tag="rrh")
                    nc.vector.tensor_scalar_add(rrh, rrf, 0.0)
                    drr = drows.tile([1, 2048], BF16, tag="drr")
                    nc.gpsimd.dma_start(out=drr, in_=rrh)
                    bc12 = renorm.tile([64, 2048], BF16, tag="bc12")
                    nc.sync.dma_start(
                        out=bc12, in_=drr.to_broadcast((64, 2048)))
                    u1 = renorm.tile([64, 1024], BF16, tag="u1")
                    nc.vector.tensor_mul(u1, smain[0:64, :], bc12[:, 0:1024])
                    u2 = renorm.tile([64, 1024], BF16, tag="u2")
                    nc.vector.tensor_mul(u2, pmem[0:64, :], bc12[:, 1024:2048])
                    nc.vector.tensor_add(attn_n[n][ho:ho + 64, :], u1, u2)

                if dbg:
                    nc.sync.dma_start(
                        out=dbg_t["dbg_q"][:, n * L:(n + 1) * L], in_=qT_n[n])
                    nc.sync.dma_start(
                        out=dbg_t["dbg_k"][:, n * L:(n + 1) * L], in_=kT_n[n])
                    nc.sync.dma_start(
                        out=dbg_t["dbg_attn"][:, n * L:(n + 1) * L],
                        in_=attn_n[n])

                # ---- out_proj partial for batch n ----
                for oc in range(8):
                    for rt2 in range(2):
                        po = pproj.tile([128, 512], F32, tag="pp")
                        nc.tensor.matmul(
                            po, wo_sb[:, oc * 128:(oc + 1) * 128],
                            attn_n[n][:, rt2 * 512:(rt2 + 1) * 512],
                            start=True, stop=True)
                        so = ostage.tile([128, 512], BF16, tag="so")
                        nc.vector.tensor_scalar_add(so, po, 0.0)
                        dst = outT[oc * 128:(oc + 1) * 128,
                                   n * L + rt2 * 512:n * L + (rt2 + 1) * 512]
                        if (oc * 2 + rt2) % 2 == 0:
                            nc.sync.dma_start(out=dst, in_=so)
                        else:
                            nc.gpsimd.dma_start(out=dst, in_=so)

            emit_proj(0)
            wo_sb = const.tile([DC, E], BF16)
            nc.scalar.dma_start(out=wo_sb, in_=woT)
            kmem_sb = const.tile([DC, N, M], BF16)
            nc.gpsimd.dma_start(out=kmem_sb, in_=kmem)
            vmaug_sb = const.tile([128, N, HPC, 4, 65], BF16)
            nc.scalar.dma_start(out=vmaug_sb, in_=vmaug)
            for n in range(N):
                if n + 1 < N:
                    emit_proj(n + 1)
                emit_attn_out(n)
    nc.compile()
    return nc


def _perm64():
    p = np.empty(64, np.int64)
    p[:32] = np.arange(0, 64, 2)
    p[32:] = np.arange(1, 64, 2)
    return p


def _prep_inputs(inputs):
    """Host-side shard prep. Returns list of per-core input dicts."""
    f = np.float32
    query = np.asarray(inputs["query"], f)
    key = np.asarray(inputs["key"], f)
    value = np.asarray(inputs["value"], f)
    W = np.asarray(inputs["in_proj_weight"], f)
    wo = np.asarray(inputs["out_proj_weight"], f)
    qp = np.asarray(inputs["qp"], f)
    kvp = np.asarray(inputs["kvp"], f)
    k_mem = np.asarray(inputs["k_mem"], f)
    v_mem = np.asarray(inputs["v_mem"], f)
    gate = np.asarray(inputs["gate_attn"], f)
    mask = np.asarray(inputs["mem_mask"]).astype(f)

    g = 1.0 / (1.0 + np.exp(-gate))
    perm64 = _perm64()
    sgn = np.concatenate([np.full(32, -1.0, f), np.full(32, 1.0, f)] * HPC)

    xqT = np.ascontiguousarray(
        query.transpose(2, 1, 0).reshape(E, R)).astype(NPBF)
    xkT = np.ascontiguousarray(
        key.transpose(2, 1, 0).reshape(E, R)).astype(NPBF)
    xvT = np.ascontiguousarray(
        value.transpose(2, 1, 0).reshape(E, R)).astype(NPBF)

    in_maps = []
    for c in range(NC):
        dims = np.arange(c * DC, (c + 1) * DC)
        dims_perm = np.concatenate([dims[h * 64 + perm64] for h in range(HPC)])
        gv = np.concatenate(
            [np.full(64, 1.0 - g[2 * c + h], f) for h in range(HPC)])

        wq = W[:E][dims_perm] * np.float32(D ** -0.5)
        wk = W[E:2 * E][dims_perm]
        wv = W[2 * E:][dims] * gv[:, None]

        def rope(pe):
            cosT = np.ascontiguousarray(
                pe[:, :, dims_perm, 0].transpose(2, 0, 1).reshape(DC, R))
            sinT = (pe[:, :, dims_perm, 1].transpose(2, 0, 1).reshape(DC, R)
                    * sgn[:, None])
            # device writes z into the partner rows, so the sin tensor must
            # be pre-swapped: st[p] = sin_signed[partner(p)]
            sw = np.empty_like(sinT)
            for hb in range(HPC):
                b = hb * 64
                sw[b:b + 32] = sinT[b + 32:b + 64]
                sw[b + 32:b + 64] = sinT[b:b + 32]
            return cosT.astype(NPBF), np.ascontiguousarray(sw).astype(NPBF)

        cq, sq = rope(qp)
        ck, sk = rope(kvp)

        kmemT = np.ascontiguousarray(
            k_mem[:, dims_perm, :].transpose(1, 0, 2)).astype(NPBF)

        vma = np.zeros((N, HPC, M, 65), f)
        for n in range(N):
            for h in range(HPC):
                gh = g[2 * c + h]
                vm = v_mem[n, dims[h * 64:(h + 1) * 64], :].T  # (M, 64)
                vma[n, h, :, :64] = vm * gh * mask[n][:, None]
                vma[n, h, :, 64] = mask[n]
        vma_dev = np.ascontiguousarray(
            vma.reshape(N, HPC, 4, 128, 65).transpose(3, 0, 1, 2, 4)).astype(NPBF)

        in_maps.append({
            "xqT": xqT, "xkT": xkT, "xvT": xvT,
            "wqT": np.ascontiguousarray(wq.T).astype(NPBF),
            "wkT": np.ascontiguousarray(wk.T).astype(NPBF),
            "wvT": np.ascontiguousarray(wv.T).astype(NPBF),
            "woT": np.ascontiguousarray(wo[:, dims].T).astype(NPBF),
            "cosq": cq, "sinq": sq, "cosk": ck, "sink": sk,
            "kmem": kmemT, "vmaug": vma_dev,
        })
    return in_maps


def kernel(**inputs):
    if "nc" not in _COMPILED:
        _COMPILED["nc"] = _build()
    nc = _COMPILED["nc"]
    in_maps = _prep_inputs(inputs)
    res = bass_utils.run_bass_kernel_spmd(nc, in_maps, core_ids=list(range(NC)))
    total = np.zeros((E, R), np.float64)
    for r in res.results:
        total += r["outT"].astype(np.float64)
    out = total.T.reshape(N, L, E).transpose(1, 0, 2).astype(np.float32)
    out = out + np.asarray(inputs["out_proj_bias"], np.float32)
    return out
